# revision 22
# baseline (speedup 1.0000x reference)
"""Trainium2 Bass kernel for nn_AttentionModule (sparse_attention).

Computation (reference):
    q = tanh(einsum("hde,be->hbd", Query, x))
    k = tanh(einsum("hde,ble->hbld", Key, bank))
    score = einsum("hbld,hbd->hbl", k, q);  masked softmax over l
    emb = einsum("hbl,ble->hbe", attn, bank);  LeakyReLU(0.4)

Sharding: head dim H=128 split across 8 cores (16 heads each), SPMD, no
collectives; output gathered host-side. Weights are pre-transposed to
(e, d) layout on the host so the contraction dim lands on partitions
without any on-chip transposes.

Per-core dataflow (matmul operands fp16, PSUM accumulate fp32):
  kproj:  psk[d,bl]     += keyT[h,ec].T @ bankT[ec]   (N=512, the FLOP floor)
  ktanh (ACT tanh, fused PSUM drain, fp16 out)
  qproj:  psq[b,(h4,d)] += xT[ec].T @ queryT[ec]      (lhsT = xT: cheap weights)
  qtanh -> PE-transpose per head -> q_all[d,h,b]
  cross:  psx[b',(b,l)]  = q_all[:,h,:].T @ ktanh     (score = diag blocks)
  exp fused into the cross drain (tanh-bounded scores cannot overflow exp,
          so no max-subtraction is needed); the diagonal blocks are pulled
          out with a small SWDGE DMA gather (partition step = pitch+64)
  softmax tail per 4-head group (overlaps next group's kproj): mask as a
          0/1 multiply, seg-sum, reciprocal, normalize (all DVE)
  emb:    attn PE-transposed to (l,b)-planes; odd-b half DMA-shifted to
          partitions 64..127 of a block-diagonal lhsT per b-pair; 2 pairs
          per (64,768) psum tile at bases {0,32}; LeakyReLU(0.4) built as
          0.7*y + 0.3*|y| (ACT Lrelu alpha is table-baked) with one wide
          drain + one contiguous DMA per 4 b's
"""

import numpy as np
from contextlib import ExitStack

import concourse.bacc as bacc
import concourse.bass as bass
import concourse.tile as tile
from concourse import mybir
from concourse.bass_utils import run_bass_kernel_spmd
from concourse.masks import make_identity

B, L, E, H, D = 16, 64, 768, 128, 128
NCORES = 8
HL = H // NCORES          # 16 heads per core
EC = E // 128             # 6 contraction chunks
NHG = HL // 4             # 4 head-groups of 4 (qproj N=512 packing)
BL = B * L                # 1024
f32 = mybir.dt.float32
MMDT = mybir.dt.float16  # matmul operand dtype (1 cyc/col on PE, separate LDW)
NPDT = mybir.dt.np(MMDT)


def build_nc():
    nc = bacc.Bacc("TRN2", target_bir_lowering=False, debug=False)

    qt = nc.dram_tensor("qt", (NHG, 128, EC, 4, D), MMDT, kind="ExternalInput").ap()
    kt = nc.dram_tensor("kt", (NHG, 128, EC, 4, D), MMDT, kind="ExternalInput").ap()
    bt = nc.dram_tensor("bt", (128, EC, BL), MMDT, kind="ExternalInput").ap()
    bk = nc.dram_tensor("bk", (128, B // 2, E), MMDT, kind="ExternalInput").ap()
    xt = nc.dram_tensor("xt", (128, EC, B), MMDT, kind="ExternalInput").ap()
    mk = nc.dram_tensor("mk", (B, 4, L), f32, kind="ExternalInput").ap()
    out = nc.dram_tensor("out", (B, HL, E), f32, kind="ExternalOutput").ap()

    with tile.TileContext(nc) as tc, ExitStack() as ctx:
        const = ctx.enter_context(tc.tile_pool(name="const", bufs=1))
        wqp = ctx.enter_context(tc.tile_pool(name="wq", bufs=2))
        wkp = ctx.enter_context(tc.tile_pool(name="wk", bufs=2))
        big = ctx.enter_context(tc.tile_pool(name="big", bufs=1))
        ktp = ctx.enter_context(tc.tile_pool(name="ktan", bufs=6))
        sm = ctx.enter_context(tc.tile_pool(name="small", bufs=1))
        stg = ctx.enter_context(tc.tile_pool(name="stage", bufs=2))
        scp = ctx.enter_context(tc.tile_pool(name="scoreC", bufs=3))
        psk = ctx.enter_context(tc.tile_pool(name="psk", bufs=2, space="PSUM"))
        pss = ctx.enter_context(tc.tile_pool(name="pss", bufs=2, space="PSUM"))

        ident = const.tile([128, 128], f32)
        make_identity(nc, ident)
        xt_sb = const.tile([128, EC, B], MMDT)
        mk_sb = const.tile([B, 4, L], f32)
        bt_sb = big.tile([128, EC, BL], MMDT)
        bk_sb = big.tile([128, B // 2, E], MMDT)

        # ---- DMAs in consumption order (kproj first, bk late)
        wq_tiles, wk_tiles = [], []
        wk_t0 = wkp.tile([128, EC, 4, D], MMDT, tag="wk")
        wk_tiles.append(wk_t0)
        # first chunk split in halves so head 0 can start ASAP
        nc.sync.dma_start(out=wk_t0[:, :, 0:2, :], in_=kt[0][:, :, 0:2, :])
        nc.sync.dma_start(out=bt_sb[:, 0, :], in_=bt[:, 0, :])
        nc.sync.dma_start(out=xt_sb, in_=xt)
        nc.sync.dma_start(out=mk_sb, in_=mk)
        nc.sync.dma_start(out=bt_sb[:, 1, :], in_=bt[:, 1, :])
        nc.sync.dma_start(out=wk_t0[:, :, 2:4, :], in_=kt[0][:, :, 2:4, :])
        for ec in range(2, EC):
            nc.sync.dma_start(out=bt_sb[:, ec, :], in_=bt[:, ec, :])
        wq_t0 = wqp.tile([128, EC, 4, D], MMDT, tag="wq")
        wq_tiles.append(wq_t0)
        nc.sync.dma_start(out=wq_t0, in_=qt[0])
        for hg in range(1, NHG):
            wk_t = wkp.tile([128, EC, 4, D], MMDT, tag="wk")
            wk_tiles.append(wk_t)
            nc.sync.dma_start(out=wk_t, in_=kt[hg])
            wq_t = wqp.tile([128, EC, 4, D], MMDT, tag="wq")
            wq_tiles.append(wq_t)
            nc.sync.dma_start(out=wq_t, in_=qt[hg])
        nc.sync.dma_start(out=bk_sb, in_=bk)

        q_all = sm.tile([128, HL, B], MMDT)      # [d, h, b]
        exu = sm.tile([B, HL, L], f32)           # [b, h, l] raw exp(score)
        exm = sm.tile([B, HL, L], f32)           # masked exp
        attn = sm.tile([B, HL, L], f32)          # normalized attention weights
        sumexp = sm.tile([B, HL], f32)
        recip = sm.tile([B, HL], f32)
        exuT_bd = sm.tile([128, B // 2, 2, HL], MMDT)  # block-diag lhsT per pair
        oddT = sm.tile([64, B // 2, HL], MMDT)         # odd-b staging
        # zero the off-diagonal blocks (top/odd and bottom/even) once
        nc.vector.memset(exuT_bd[0:64, :, 1, :], 0.0)
        nc.vector.memset(exuT_bd[64:128, :, 0, :], 0.0)

        # ---- main loop: kproj+tanh for 4 heads, then qproj(hg), then crosses
        ktan_tiles = {}
        for hg in range(NHG):
            for hl4 in range(4):
                h = hg * 4 + hl4
                psk_t = psk.tile([128, BL], f32)
                for ec in range(EC):
                    for nc5 in range(2):
                        nc.tensor.matmul(
                            psk_t[:, nc5 * 512:(nc5 + 1) * 512],
                            wk_tiles[hg][:, ec, hl4, :],
                            bt_sb[:, ec, nc5 * 512:(nc5 + 1) * 512],
                            start=(ec == 0), stop=(ec == EC - 1),
                        )
                ktan = ktp.tile([128, BL], MMDT)
                nc.scalar.activation(out=ktan, in_=psk_t,
                                     func=mybir.ActivationFunctionType.Tanh)
                ktan_tiles[h] = ktan

            # qproj for this head group
            psq = pss.tile([B, 4, D], f32, tag="px")
            for ec in range(EC):
                nc.tensor.matmul(
                    psq.rearrange("b h4 d -> b (h4 d)"),
                    xt_sb[:, ec, :],
                    wq_tiles[hg][:, ec, :, :].rearrange("p h4 d -> p (h4 d)"),
                    start=(ec == 0), stop=(ec == EC - 1),
                )
            qtan = sm.tile([B, 4, D], f32, tag=f"qtan{hg}")
            nc.scalar.activation(out=qtan, in_=psq,
                                 func=mybir.ActivationFunctionType.Tanh)
            for h4 in range(4):
                pst = pss.tile([128, B], f32, tag="px")
                nc.tensor.transpose(pst, qtan[:, h4, :], ident[0:B, 0:B])
                nc.vector.tensor_copy(out=q_all[:, hg * 4 + h4, :], in_=pst)

            # score cross + diagonal extraction for the 4 heads
            for hl4 in range(4):
                h = hg * 4 + hl4
                psx = pss.tile([B, BL], f32, tag="px")
                for nc5 in range(2):
                    nc.tensor.matmul(
                        psx[:, nc5 * 512:(nc5 + 1) * 512],
                        q_all[:, h, :],
                        ktan_tiles[h][:, nc5 * 512:(nc5 + 1) * 512],
                        start=True, stop=True,
                    )
                scoreC = scp.tile([B, BL], f32)
                nc.scalar.activation(out=scoreC, in_=psx,
                                     func=mybir.ActivationFunctionType.Exp)
                cfull = scoreC[:, :]
                diag = bass.AP(tensor=cfull.tensor, offset=cfull.offset,
                               ap=[[cfull.ap[0][0] + L, B], [1, L]])
                nc.gpsimd.dma_start(out=exu[:, h, :], in_=diag)

            # softmax tail for this head group (overlaps next group's kproj);
            # exp already fused into the cross drain, mask is multiplicative
            hs4 = slice(hg * 4, hg * 4 + 4)
            nc.vector.tensor_tensor(out=exm[:, hs4, :], in0=exu[:, hs4, :],
                                    in1=mk_sb, op=mybir.AluOpType.mult)
            nc.vector.tensor_reduce(out=sumexp[:, hs4], in_=exm[:, hs4, :],
                                    axis=mybir.AxisListType.X,
                                    op=mybir.AluOpType.add)
            nc.vector.reciprocal(out=recip[:, hs4], in_=sumexp[:, hs4])
            nc.vector.tensor_tensor(out=attn[:, hs4, :], in0=exm[:, hs4, :],
                                    in1=recip[:, hs4].broadcast_to((B, 4, L)),
                                    op=mybir.AluOpType.mult)

            # attn -> (l, b)-planes for this group (transpose out must be at
            # psum partition 0); even b -> top half of the block-diag lhsT,
            # odd b staged in SBUF then DMA-shifted to partitions 64..127
            psxT = pss.tile([64, 4, B], f32, tag="px")
            for h4 in range(4):
                h = hg * 4 + h4
                nc.tensor.transpose(psxT[:, h4, :], attn[:, h, :],
                                    ident[0:B, 0:B])
            nc.vector.tensor_copy(
                out=exuT_bd[0:64, :, 0, hs4],
                in_=psxT.rearrange("p h b -> p b h")[:, 0::2, :])
            nc.vector.tensor_copy(
                out=oddT[:, :, hs4],
                in_=psxT.rearrange("p h b -> p b h")[:, 1::2, :])
            nc.gpsimd.dma_start(out=exuT_bd[64:128, :, 1, hs4],
                                in_=oddT[:, :, hs4])

        # ---- emb per pair of b (psum bases {0,32}), fused LeakyReLU drain
        # per 4 b's. LeakyReLU(0.4) == 0.7*y + 0.3*|y| (the ACT Lrelu alpha
        # is table-baked, so build it from Abs + Copy).
        for g in range(B // 4):
            pse = pss.tile([64, E], f32, tag="px")
            for j in range(2):
                p = 2 * g + j
                for n0, n1 in ((0, 512), (512, 768)):
                    nc.tensor.matmul(
                        pse[j * 32:j * 32 + 32, n0:n1],
                        exuT_bd[:, p, :, :].rearrange("p a h -> p (a h)"),
                        bk_sb[:, p, n0:n1],
                        start=True, stop=True,
                    )
            t3a = stg.tile([64, E], f32, tag="t3a")
            nc.scalar.activation(out=t3a, in_=pse,
                                 func=mybir.ActivationFunctionType.Abs,
                                 bias=0.0, scale=0.3)
            t7 = stg.tile([64, E], f32, tag="t7")
            nc.scalar.mul(t7, pse, 0.7)
            stage = stg.tile([64, E], f32)
            nc.vector.tensor_tensor(out=stage, in0=t7, in1=t3a,
                                    op=mybir.AluOpType.add)
            nc.sync.dma_start(
                out=out[4 * g:4 * g + 4].rearrange("b h e -> (b h) e"),
                in_=stage)

    nc.compile()
    return nc


def prep_core_inputs(x, bank, mask, Query, Key, core):
    """Host-side shard + relayout for one core (not in the HW-timed window)."""
    hs = slice(core * HL, (core + 1) * HL)

    # [hg, p, ec, h4, d] with element = W[hg*4+h4, d, ec*128+p]
    def wT(W):
        A = W[hs].transpose(2, 0, 1)                      # [e, h, d]
        A = A.reshape(EC, 128, NHG, 4, D)                 # [ec, p, hg, h4, d]
        return np.ascontiguousarray(A.transpose(2, 1, 0, 3, 4)).astype(NPDT)

    bankf = bank.reshape(BL, E)
    bt = np.ascontiguousarray(
        bankf.T.reshape(EC, 128, BL).transpose(1, 0, 2)).astype(NPDT)
    bk = np.ascontiguousarray(
        bank.reshape(B // 2, 2, L, E).transpose(1, 2, 0, 3).reshape(128, B // 2, E)
    ).astype(NPDT)
    xt = np.ascontiguousarray(
        x.T.reshape(EC, 128, B).transpose(1, 0, 2)).astype(NPDT)
    m01 = mask.astype(np.float32)
    mk = np.ascontiguousarray(np.repeat(m01[:, None, :], 4, axis=1))
    return {"qt": wT(Query), "kt": wT(Key), "bt": bt, "bk": bk, "xt": xt, "mk": mk}


def kernel(x, bank, mask, Query, Key):
    x = np.asarray(x, dtype=np.float32)
    bank = np.asarray(bank, dtype=np.float32)
    mask_np = np.asarray(mask)
    Query = np.asarray(Query, dtype=np.float32)
    Key = np.asarray(Key, dtype=np.float32)

    nc = build_nc()
    in_maps = [prep_core_inputs(x, bank, mask_np, Query, Key, c)
               for c in range(NCORES)]
    res = run_bass_kernel_spmd(nc, in_maps, list(range(NCORES)))
    out = np.empty((B, H, E), dtype=np.float32)
    for c in range(NCORES):
        out[:, c * HL:(c + 1) * HL, :] = res.results[c]["out"]
    return out


if __name__ == "__main__":
    rng = np.random.default_rng(0)
    ins = {
        "x": rng.standard_normal((B, E), dtype=np.float32),
        "bank": rng.standard_normal((B, L, E), dtype=np.float32),
        "mask": rng.integers(0, 2, size=(B, L)).astype(np.int32),
        "Query": rng.standard_normal((H, D, E), dtype=np.float32) * 0.05,
        "Key": rng.standard_normal((H, D, E), dtype=np.float32) * 0.05,
    }
    print(kernel(**ins).shape)
